# revision 1
# baseline (speedup 1.0000x reference)
"""Trainium2 Bass kernel for nn_MultiHeadAttention_72189810312078.

Computation (per token): qkv = x @ w_qkv.T + b_qkv; per-token attention over
the 16 heads with 16x16 score matrices; out = attn_out @ w_out.T + b_out.

Strategy: data-parallel over 8 NeuronCores (8192 tokens each). Host
pre-transposes x to xT [1024, N] so the channel (contraction) dim lands on
SBUF partitions. Per core, per 256-token superblock:
  1) qkvT projection: 24 feature-chunk matmuls (N=256), K=1024 accumulated in
     PSUM; per-partition bias added in PSUM with one tensor_scalar_add.
  2) PSUM chunks scatter-evicted into attention staging: Q at partitions
     64:128 of T1, K at 64:128 / V at 0:64 of T2 (matmul operands need equal
     base partitions), laid out [d, (group, head, t)].
  3) Attention in groups of 8 tokens ((g,t) packs 16x8=128 partitions):
     scoresT = K.T @ Q per group (K=64 matmul at tile_position row 64);
     exp on ScalarE; multiplicative block-diagonal mask; V8 =
     PE-transpose(V); attnV matmul with a ones column appended to V8 so the
     softmax denominator falls out of the same matmul; normalize with a
     per-partition reciprocal scale on eviction.
  4) attn output PE-transposed back to feature-major, packed into S2
     [128 = (dlt,d), chunk x token]; out-projection against host-permuted
     w_out.T rows (feature 64*(8*dlt+c)+d at S2 row 128c+64*dlt+d); bias
     added from a replicated tile during eviction; result DMA'd row-major.

Dtype mode: projections can run in float32r (fp32 rounded to 11 mantissa
bits, 4x faster on the PE at N>=256) or exact float32. Attention
scores/attnV always accumulate in fp32 PSUM.
"""

import os
import sys
from contextlib import ExitStack

sys.path.insert(0, "/opt/trn_rl_repo")

import numpy as np

import concourse.bass as bass  # noqa: E402
import concourse.bacc as bacc  # noqa: E402
import concourse.tile as tile  # noqa: E402
from concourse import mybir  # noqa: E402
from concourse.bass_utils import run_bass_kernel_spmd  # noqa: E402
from concourse.masks import make_identity  # noqa: E402

F32 = mybir.dt.float32
F32R = mybir.dt.float32r

N_CORES = 8
H, D, C = 16, 64, 1024
SB = 256   # tokens per superblock (projection moving dim)
SS = 128   # tokens per attention sub-stage / out-projection block
NG = SB // 8   # token groups per superblock (32)

USE_F32R = os.environ.get("KMODE", "f32r") == "f32r"
GPS = int(os.environ.get("GPS", "3"))  # bitmask: 1=memset, 2=mask-mul, 4=bias
Exp = mybir.ActivationFunctionType.Exp
Copy = mybir.ActivationFunctionType.Copy


def build(tok, use_f32r=USE_F32R, static_loop=False):
    WD = F32R if use_f32r else F32   # projection operand dtype
    AD = F32R if use_f32r else F32   # attn-out / S2 dtype (out-proj lhsT)

    nc = bacc.Bacc("TRN2", target_bir_lowering=False, debug=False,
                   enable_asserts=True, num_devices=N_CORES)
    xT_d = nc.dram_tensor("xT", [C, tok], WD, kind="ExternalInput").ap()
    wqkvT_d = nc.dram_tensor("wqkvT", [C, 3 * C], WD, kind="ExternalInput").ap()
    woutT_d = nc.dram_tensor("woutT", [C, C], WD, kind="ExternalInput").ap()
    bcols_d = nc.dram_tensor("bcols", [128, 24], F32, kind="ExternalInput").ap()
    borep_d = nc.dram_tensor("borep", [128, C], F32, kind="ExternalInput").ap()
    maskB_d = nc.dram_tensor("maskB", [128, 512], F32, kind="ExternalInput").ap()
    out_d = nc.dram_tensor("out", [tok, C], F32, kind="ExternalOutput").ap()

    with tile.TileContext(nc) as tc, ExitStack() as ctx:
        consts = ctx.enter_context(tc.tile_pool(name="consts", bufs=1))
        xin = ctx.enter_context(tc.tile_pool(name="xin", bufs=2))
        stag = ctx.enter_context(tc.tile_pool(name="stag", bufs=1))
        smx = ctx.enter_context(tc.tile_pool(name="smx", bufs=2))
        s2p = ctx.enter_context(tc.tile_pool(name="s2p", bufs=2))
        outp = ctx.enter_context(tc.tile_pool(name="outp", bufs=2))
        psA = ctx.enter_context(tc.tile_pool(name="psA", bufs=2, space="PSUM"))
        psSp = ctx.enter_context(tc.tile_pool(name="psSp", bufs=2, space="PSUM"))
        psVp = ctx.enter_context(tc.tile_pool(name="psVp", bufs=1, space="PSUM"))
        psC2p = ctx.enter_context(tc.tile_pool(name="psC2p", bufs=1, space="PSUM"))
        psTp = ctx.enter_context(tc.tile_pool(name="psTp", bufs=1, space="PSUM"))
        psOp = ctx.enter_context(tc.tile_pool(name="psOp", bufs=1, space="PSUM"))

        # ---- constants ----
        wq_sb = consts.tile([128, 8, 3 * C], WD)
        nc.sync.dma_start(out=wq_sb, in_=wqkvT_d.rearrange("(ci p) f -> p ci f", p=128))
        wo_sb = consts.tile([128, 8, C], WD)
        nc.sync.dma_start(out=wo_sb, in_=woutT_d.rearrange("(ci p) f -> p ci f", p=128))
        bcols_sb = consts.tile([128, 24], F32)
        nc.sync.dma_start(out=bcols_sb, in_=bcols_d)
        borep_sb = consts.tile([128, C], F32)
        nc.sync.dma_start(out=borep_sb, in_=borep_d)
        maskB_sb = consts.tile([128, 512], F32)
        nc.sync.dma_start(out=maskB_sb, in_=maskB_d)
        idq = consts.tile([128, 128], F32)
        make_identity(nc, idq)
        if AD is F32:
            idr = idq
        else:
            idr = consts.tile([128, 128], AD)
            nc.vector.tensor_copy(idr, idq)

        ecnt = 0  # evict-engine round robin

        def evict_copy(dst, src):
            nonlocal ecnt
            if ecnt % 2 == 0:
                nc.vector.tensor_copy(dst, src)
            else:
                nc.scalar.copy(dst, src)
            ecnt += 1

        xT_r = xT_d.rearrange("(ci p) t -> p ci t", p=128)
        from contextlib import nullcontext
        if static_loop:
            loop_iter = [(nullcontext(iv), iv) for iv in range(0, tok, SB)]
        else:
            fc = tc.For_i(0, tok, SB,
                          hint_engines=(mybir.EngineType.PE,
                                        mybir.EngineType.DVE))
            loop_iter = [(fc, None)]
        for _ctx, _iv in loop_iter:
          with _ctx as _cv:
            iv = _iv if _iv is not None else _cv
            x_sb = xin.tile([128, 8, SB], WD)
            nc.sync.dma_start(out=x_sb, in_=xT_r[:, :, bass.ds(iv, SB)])

            # staging: T1 rows 64:128 = Q; T2 rows 64:128 = K, rows 0:64 = V
            T1 = stag.tile([128, NG, 16, 8], F32, name="T1")
            T2 = stag.tile([128, NG, 16, 8], F32, name="T2")

            # ---- qkv projection + scatter-evict (bias fused / on gpsimd) ----
            for co in range(24):
                psC1 = psA.tile([128, SB], F32)
                for ci in range(8):
                    nc.tensor.matmul(psC1, wq_sb[:, ci, co * 128:(co + 1) * 128],
                                     x_sb[:, ci, :], start=(ci == 0),
                                     stop=(ci == 7))
                kind, c = co // 8, co % 8
                for dlt in range(2):
                    src = psC1[64 * dlt:64 * dlt + 64, :].rearrange(
                        "p (g t) -> p g t", g=NG)
                    hslot = 2 * c + dlt
                    if kind == 0:
                        dst = T1[64:128, :, hslot, :]
                    elif kind == 1:
                        dst = T2[64:128, :, hslot, :]
                    else:
                        dst = T2[0:64, :, hslot, :]
                    bias = bcols_sb[64 * dlt:64 * dlt + 64, co:co + 1]
                    if dlt == 0:
                        # DVE evict with fused bias add
                        nc.vector.tensor_scalar_add(dst, src, bias)
                    elif GPS & 4:
                        # ACT plain evict, bias added SBUF-side on idle gpsimd
                        nc.scalar.copy(dst, src)
                        nc.gpsimd.tensor_scalar_add(dst, dst, bias)
                    else:
                        nc.vector.tensor_scalar_add(dst, src, bias)

            # ---- attention (8 batches of 4 groups) + out-proj per 128 tok ----
            for iss in range(2):
                S2 = s2p.tile([128, 8, SS], AD)
                for b4 in range(4 * iss, 4 * iss + 4):
                    psS = psSp.tile([128, 512], F32)
                    psV = psVp.tile([128, 4, 64], F32)
                    for j in range(4):
                        g = 4 * b4 + j
                        nc.tensor.matmul(psS[:, 128 * j:128 * j + 128],
                                         T2[64:128, g, :, :], T1[64:128, g, :, :],
                                         start=True, stop=True)
                        nc.tensor.transpose(psV[:, j, :], T2[0:64, g, :, :],
                                            idq[0:64, 0:64])
                    es4 = smx.tile([128, 512], F32)
                    nc.scalar.activation(es4, psS, Exp, scale=0.125)
                    if GPS & 2:
                        nc.gpsimd.tensor_mul(es4, es4, maskB_sb)
                    else:
                        nc.vector.tensor_mul(es4, es4, maskB_sb)
                    V8sb = smx.tile([128, 4, 66], F32)
                    nc.scalar.copy(V8sb[:, :, 0:64], psV)
                    if GPS & 1:
                        nc.gpsimd.memset(V8sb[:, :, 64:65], 1.0)
                    else:
                        nc.vector.memset(V8sb[:, :, 64:65], 1.0)
                    psC2 = psC2p.tile([128, 4, 66], F32)
                    for j in range(4):
                        nc.tensor.matmul(psC2[:, j, 0:65],
                                         es4[:, 128 * j:128 * j + 128],
                                         V8sb[:, j, 0:65], start=True, stop=True)
                    rec4 = smx.tile([128, 4], F32)
                    nc.vector.reciprocal(rec4, psC2[:, :, 64:65])
                    attno = smx.tile([128, 4, 64], AD)
                    for j in range(4):
                        if j % 2 == 0:
                            nc.scalar.activation(attno[:, j, :], psC2[:, j, 0:64],
                                                 Copy, scale=rec4[:, j:j + 1])
                        else:
                            nc.vector.tensor_scalar_mul(attno[:, j, :],
                                                        psC2[:, j, 0:64],
                                                        rec4[:, j:j + 1])
                    psT = psTp.tile([64, 4, 128], AD)
                    for j in range(4):
                        nc.tensor.transpose(psT[:, j, :], attno[:, j, :], idr)
                    # S2 pack: head slots 8*dlt..8*dlt+7 -> S2 rows 64*dlt+d
                    for dlt in range(2):
                        src = psT[:, :, 64 * dlt:64 * dlt + 64].rearrange(
                            "p j (h t) -> p j h t", h=8)
                        dst = S2[64 * dlt:64 * dlt + 64].rearrange(
                            "p c (gb gj t) -> p gj c gb t", gb=4, gj=4)[
                                :, :, :, b4 % 4, :]
                        evict_copy(dst, src)

                # ---- out-projection for this 128-token block ----
                outsb = outp.tile([128, C], F32)
                for nh in range(2):
                    psO = psOp.tile([128, 512], F32)
                    for c in range(8):
                        nc.tensor.matmul(psO, S2[:, c, :],
                                         wo_sb[:, c, 512 * nh:512 * nh + 512],
                                         start=(c == 0), stop=(c == 7))
                    nc.vector.tensor_add(outsb[:, 512 * nh:512 * nh + 512], psO,
                                         borep_sb[:, 512 * nh:512 * nh + 512])
                nc.sync.dma_start(out=out_d[bass.ds(iv + SS * iss, SS), :],
                                  in_=outsb)

    nc.compile()
    return nc


def build_pipe(tok, use_f32r=USE_F32R):
    """Software-pipelined build: attention of superblock k overlaps the
    projection of superblock k+1 inside one For_i body (2 superblocks per
    iteration, ping-pong staging halves packed into shared tiles)."""
    WD = F32R if use_f32r else F32
    AD = F32R if use_f32r else F32

    nc = bacc.Bacc("TRN2", target_bir_lowering=False, debug=False,
                   enable_asserts=True, num_devices=N_CORES)
    xT_d = nc.dram_tensor("xT", [C, tok], WD, kind="ExternalInput").ap()
    wqkvT_d = nc.dram_tensor("wqkvT", [C, 3 * C], WD, kind="ExternalInput").ap()
    woutT_d = nc.dram_tensor("woutT", [C, C], WD, kind="ExternalInput").ap()
    bcols_d = nc.dram_tensor("bcols", [128, 24], F32, kind="ExternalInput").ap()
    borep_d = nc.dram_tensor("borep", [128, C], F32, kind="ExternalInput").ap()
    maskB_d = nc.dram_tensor("maskB", [128, 512], F32, kind="ExternalInput").ap()
    out_d = nc.dram_tensor("out", [tok, C], F32, kind="ExternalOutput").ap()

    with tile.TileContext(nc) as tc, ExitStack() as ctx:
        consts = ctx.enter_context(tc.tile_pool(name="consts", bufs=1))
        xin = ctx.enter_context(tc.tile_pool(name="xin", bufs=1))
        stag = ctx.enter_context(tc.tile_pool(name="stag", bufs=1))
        smx = ctx.enter_context(tc.tile_pool(name="smx", bufs=1))
        s2p = ctx.enter_context(tc.tile_pool(name="s2p", bufs=1))
        outp = ctx.enter_context(tc.tile_pool(name="outp", bufs=1))
        psA = ctx.enter_context(tc.tile_pool(name="psA", bufs=2, space="PSUM"))
        psSp = ctx.enter_context(tc.tile_pool(name="psSp", bufs=2, space="PSUM"))
        psVp = ctx.enter_context(tc.tile_pool(name="psVp", bufs=1, space="PSUM"))
        psC2p = ctx.enter_context(tc.tile_pool(name="psC2p", bufs=1, space="PSUM"))
        psTp = ctx.enter_context(tc.tile_pool(name="psTp", bufs=1, space="PSUM"))
        psOp = ctx.enter_context(tc.tile_pool(name="psOp", bufs=1, space="PSUM"))

        wq_sb = consts.tile([128, 8, 3 * C], WD)
        nc.sync.dma_start(out=wq_sb, in_=wqkvT_d.rearrange("(ci p) f -> p ci f", p=128))
        wo_sb = consts.tile([128, 8, C], WD)
        nc.sync.dma_start(out=wo_sb, in_=woutT_d.rearrange("(ci p) f -> p ci f", p=128))
        bcols_sb = consts.tile([128, 24], F32)
        nc.sync.dma_start(out=bcols_sb, in_=bcols_d)
        borep_sb = consts.tile([128, C], F32)
        nc.sync.dma_start(out=borep_sb, in_=borep_d)
        maskB_sb = consts.tile([128, 512], F32)
        nc.sync.dma_start(out=maskB_sb, in_=maskB_d)
        idq = consts.tile([128, 128], F32)
        make_identity(nc, idq)
        if AD is F32:
            idr = idq
        else:
            idr = consts.tile([128, 128], AD)
            nc.vector.tensor_copy(idr, idq)

        # persistent ping-pong staging (half-set hb=0: Q/K upper, V lower)
        Q_AB = stag.tile([128, NG, 16, 8], F32, name="Q_AB")
        K_AB = stag.tile([128, NG, 16, 8], F32, name="K_AB")
        V_AB = stag.tile([128, NG, 16, 8], F32, name="V_AB")

        xT_r = xT_d.rearrange("(ci p) t -> p ci t", p=128)

        def emit_xload(piv):
            x_sb = xin.tile([128, 8, SB], WD)
            nc.sync.dma_start(out=x_sb, in_=xT_r[:, :, bass.ds(piv, SB)])
            return x_sb

        def qk_half(T, hb):
            return T[64 * (1 - hb):64 * (1 - hb) + 64]

        def v_half(hb):
            return V_AB[64 * hb:64 * hb + 64]

        def emit_proj_chunk(x_sb, co, hb):
            psC1 = psA.tile([128, SB], F32)
            for ci in range(8):
                nc.tensor.matmul(psC1, wq_sb[:, ci, co * 128:(co + 1) * 128],
                                 x_sb[:, ci, :], start=(ci == 0), stop=(ci == 7))
            kind, c = co // 8, co % 8
            for dlt in range(2):
                src = psC1[64 * dlt:64 * dlt + 64, :].rearrange(
                    "p (g t) -> p g t", g=NG)
                hslot = 2 * c + dlt
                if kind == 0:
                    dst = qk_half(Q_AB, hb)[:, :, hslot, :]
                elif kind == 1:
                    dst = qk_half(K_AB, hb)[:, :, hslot, :]
                else:
                    dst = v_half(hb)[:, :, hslot, :]
                bias = bcols_sb[64 * dlt:64 * dlt + 64, co:co + 1]
                if dlt == 0:
                    nc.vector.tensor_scalar_add(dst, src, bias)
                else:
                    nc.scalar.copy(dst, src)
                    nc.gpsimd.tensor_scalar_add(dst, dst, bias)

        def emit_attn_batch1(b4, hb):
            """scores + V transposes + exp + mask for groups 4*b4..4*b4+3."""
            psS = psSp.tile([128, 512], F32)
            psV = psVp.tile([128, 4, 64], F32)
            vb = 64 * hb
            for j in range(4):
                g = 4 * b4 + j
                nc.tensor.matmul(psS[:, 128 * j:128 * j + 128],
                                 qk_half(K_AB, hb)[:, g, :, :],
                                 qk_half(Q_AB, hb)[:, g, :, :],
                                 start=True, stop=True)
                nc.tensor.transpose(psV[:, j, :], v_half(hb)[:, g, :, :],
                                    idq[vb:vb + 64, vb:vb + 64])
            es4 = smx.tile([128, 512], F32)
            nc.scalar.activation(es4, psS, Exp, scale=0.125)
            nc.gpsimd.tensor_mul(es4, es4, maskB_sb)
            V8sb = smx.tile([128, 4, 66], F32)
            nc.scalar.copy(V8sb[:, :, 0:64], psV)
            nc.gpsimd.memset(V8sb[:, :, 64:65], 1.0)
            return es4, V8sb

        def emit_attn_batch2(b4, es4, V8sb, S2):
            psC2 = psC2p.tile([128, 4, 66], F32)
            for j in range(4):
                nc.tensor.matmul(psC2[:, j, 0:65], es4[:, 128 * j:128 * j + 128],
                                 V8sb[:, j, 0:65], start=True, stop=True)
            rec4 = smx.tile([128, 4], F32)
            nc.vector.reciprocal(rec4, psC2[:, :, 64:65])
            attno = smx.tile([128, 4, 64], AD)
            for j in range(4):
                if j % 2 == 0:
                    nc.scalar.activation(attno[:, j, :], psC2[:, j, 0:64],
                                         Copy, scale=rec4[:, j:j + 1])
                else:
                    nc.vector.tensor_scalar_mul(attno[:, j, :],
                                                psC2[:, j, 0:64],
                                                rec4[:, j:j + 1])
            psT = psTp.tile([64, 4, 128], AD)
            for j in range(4):
                nc.tensor.transpose(psT[:, j, :], attno[:, j, :], idr)
            for dlt in range(2):
                src = psT[:, :, 64 * dlt:64 * dlt + 64].rearrange(
                    "p j (h t) -> p j h t", h=8)
                dst = S2[64 * dlt:64 * dlt + 64].rearrange(
                    "p c (gb gj t) -> p gj c gb t", gb=4, gj=4)[:, :, :, b4 % 4, :]
                if dlt == 0:
                    nc.vector.tensor_copy(dst, src)
                else:
                    nc.scalar.copy(dst, src)

        def emit_outproj(S2, oiv, iss):
            outsb = outp.tile([128, C], F32)
            for nh in range(2):
                psO = psOp.tile([128, 512], F32)
                for c in range(8):
                    nc.tensor.matmul(psO, S2[:, c, :],
                                     wo_sb[:, c, 512 * nh:512 * nh + 512],
                                     start=(c == 0), stop=(c == 7))
                nc.vector.tensor_add(outsb[:, 512 * nh:512 * nh + 512], psO,
                                     borep_sb[:, 512 * nh:512 * nh + 512])
            nc.sync.dma_start(out=out_d[bass.ds(oiv + SS * iss, SS), :], in_=outsb)

        def emit_part(attn_oiv, attn_hb, proj_piv, proj_hb):
            """Weave attention of one superblock with projection of another.
            Either may be None (prologue/epilogue)."""
            x_sb = emit_xload(proj_piv) if proj_piv is not None else None
            S2 = None
            for b4 in range(8):
                if attn_oiv is not None:
                    if b4 % 4 == 0:
                        S2 = s2p.tile([128, 8, SS], AD, name="S2")
                    pend = emit_attn_batch1(b4, attn_hb)
                if x_sb is not None:
                    for co in range(3 * b4, 3 * b4 + 3):
                        emit_proj_chunk(x_sb, co, proj_hb)
                if attn_oiv is not None:
                    emit_attn_batch2(b4, *pend, S2)
                    if b4 % 4 == 3:
                        emit_outproj(S2, attn_oiv, b4 // 4)

        assert tok % (2 * SB) == 0 and tok >= 2 * SB
        emit_part(None, None, 0, 0)                      # prologue: proj sb0 -> A
        if tok > 2 * SB:
            with tc.For_i(0, tok - 2 * SB, 2 * SB,
                          hint_engines=(mybir.EngineType.PE, mybir.EngineType.DVE,
                                        mybir.EngineType.Activation)) as iv:
                emit_part(iv, 0, iv + SB, 1)             # attn A, proj -> B
                emit_part(iv + SB, 1, iv + 2 * SB, 0)    # attn B, proj -> A
        last = tok - 2 * SB
        emit_part(last, 0, tok - SB, 1)                  # attn A, proj last -> B
        emit_part(tok - SB, 1, None, None)               # attn B

    nc.compile()
    return nc


def _round_f32r(a):
    """Round fp32 to the f32r grid (drop 12 mantissa bits, round-to-nearest)."""
    b = np.ascontiguousarray(a, dtype=np.float32).view(np.uint32)
    b = ((b + (1 << 11)) >> 12) << 12
    return b.view(np.float32)


def _host_prep(x, w_qkv, b_qkv, w_out, b_out, use_f32r=USE_F32R):
    d = np.arange(D)
    perm_q = (192 * np.arange(H)[:, None] + d[None, :]).reshape(-1)
    perm = np.concatenate([perm_q, perm_q + 64, perm_q + 128])
    wqkvT = np.ascontiguousarray(w_qkv[perm, :].T, dtype=np.float32)
    bcols = np.ascontiguousarray(
        b_qkv[perm].reshape(24, 128).T, dtype=np.float32)
    # out-proj row perm: S2 row 128c+64dlt+d holds feature 64*(8dlt+c)+d
    co, dl = np.arange(8), np.arange(2)
    perm_o = (64 * (8 * dl[None, :, None] + co[:, None, None])
              + d[None, None, :]).reshape(-1)
    woutT = np.ascontiguousarray(w_out.T[perm_o, :], dtype=np.float32)
    borep = np.ascontiguousarray(
        np.broadcast_to(b_out[None, :], (128, C)), dtype=np.float32)
    maskB = np.tile((np.arange(128)[:, None] % 8
                     == np.arange(128)[None, :] % 8).astype(np.float32), (1, 4))
    xT = np.ascontiguousarray(x.T, dtype=np.float32)
    if use_f32r:
        xT = _round_f32r(xT)
        wqkvT = _round_f32r(wqkvT)
        woutT = _round_f32r(woutT)
    return xT, wqkvT, bcols, woutT, borep, maskB


_cache = {}


def kernel(x, w_qkv, b_qkv, w_out, b_out, _trace=False, _tmpdir=None):
    x = np.asarray(x)
    n = x.shape[0]
    tok = n // N_CORES
    xT, wqkvT, bcols, woutT, borep, maskB = _host_prep(
        np.asarray(x), np.asarray(w_qkv), np.asarray(b_qkv),
        np.asarray(w_out), np.asarray(b_out))
    pipe = os.environ.get("PIPE", "1") == "1"
    key = (tok, USE_F32R, pipe)
    if key not in _cache:
        _cache[key] = build_pipe(tok) if pipe else build(tok)
    nc = _cache[key]
    shared = dict(wqkvT=wqkvT, woutT=woutT, bcols=bcols, borep=borep, maskB=maskB)
    in_maps = [dict(xT=np.ascontiguousarray(xT[:, i * tok:(i + 1) * tok]), **shared)
               for i in range(N_CORES)]
    res = run_bass_kernel_spmd(nc, in_maps, core_ids=list(range(N_CORES)),
                               trace=_trace, tmpdir=_tmpdir)
    out = np.concatenate([res.results[i]["out"] for i in range(N_CORES)], axis=0)
    kernel.last_results = res
    return out



# revision 67
# speedup vs baseline: 11621.6084x; 11621.6084x over previous
"""Trainium2 Bass kernel for nn_MultiHeadAttention_72189810312078.

Computation (per token): qkv = x @ w_qkv.T + b_qkv; per-token attention over
the 16 heads with 16x16 score matrices; out = attn_out @ w_out.T + b_out.

Strategy: data-parallel over 8 NeuronCores (8192 tokens each). Host
pre-transposes x to xT [1024, N] (bf16) so the channel (contraction) dim
lands on SBUF partitions. Per core, per 256-token superblock:
  1) qkvT projection: 24 feature-chunk matmuls (N=256), K=1024 accumulated in
     PSUM; per-partition bias fused into the eviction (DVE tensor_scalar_add
     / ACT Identity+bias).
  2) PSUM chunks scatter-evicted (bf16) into attention staging: Q at
     partitions 64:128 of T1, K at 64:128 / V at 0:64 of T2, laid out
     [d, (group, head, t)].
  3) Attention in groups of 8 tokens ((g,t) packs 16x8=128 partitions):
     scoresT = K.T @ Q per group (K=64 matmul at tile_position row 64);
     exp on ScalarE (bf16 out); multiplicative block-diagonal mask on
     GpSimd; V8 = PE-transpose(V); attnV matmul with a ones column appended
     to V8 so the softmax denominator falls out of the same matmul;
     normalized in one DVE op with the reciprocal broadcast along d.
  4) attn output PE-transposed (two groups per transpose) back to
     feature-major, packed into S2 [128 = (dlt,d), chunk x token] (bf16);
     out-projection against host-permuted w_out.T rows; bias added from a
     replicated tile during eviction; result DMA'd row-major (f32).

All SBUF operands are bf16 (PE at 1 cycle/row for any moving dim; half the
DMA and LDWEIGHTS traffic); PSUM accumulation is always fp32.
"""

import os
import sys
from contextlib import ExitStack

sys.path.insert(0, "/opt/trn_rl_repo")

import numpy as np
import ml_dtypes

import concourse.bass as bass  # noqa: E402
import concourse.bacc as bacc  # noqa: E402
import concourse.tile as tile  # noqa: E402
from concourse import mybir  # noqa: E402
from concourse.bass_utils import run_bass_kernel_spmd  # noqa: E402
from concourse.masks import make_identity  # noqa: E402

F32 = mybir.dt.float32
BF16 = mybir.dt.bfloat16

N_CORES = 8
H, D, C = 16, 64, 1024
SB = 256   # tokens per superblock (projection moving dim)
SS = 128   # tokens per attention sub-stage / out-projection block
NG = SB // 8   # token groups per superblock (32)

Exp = mybir.ActivationFunctionType.Exp
Copy = mybir.ActivationFunctionType.Copy
Identity = mybir.ActivationFunctionType.Identity

MASK_ENG = os.environ.get("MASK_ENG", "pool")  # dve | pool
UNROLL = int(os.environ.get("UNROLL", "16"))  # superblocks per For_i iteration


def build_pipe(tok):
    """Software-pipelined build: attention of superblock k overlaps the
    projection of superblock k+1 inside one For_i body (2 superblocks per
    iteration, ping-pong staging halves packed into shared tiles)."""
    WD = BF16   # projection operand dtype (x, weights)
    AD = BF16   # staging / attn-out / S2 dtype

    nc = bacc.Bacc("TRN2", target_bir_lowering=False, debug=False,
                   enable_asserts=True, num_devices=N_CORES)
    xT_d = nc.dram_tensor("xT", [C, tok], WD, kind="ExternalInput").ap()
    wqkvT_d = nc.dram_tensor("wqkvT", [C, 3 * C], WD, kind="ExternalInput").ap()
    woutT_d = nc.dram_tensor("woutT", [C, C], WD, kind="ExternalInput").ap()
    bcols_d = nc.dram_tensor("bcols", [128, 24], F32, kind="ExternalInput").ap()
    borep_d = nc.dram_tensor("borep", [128, C], F32, kind="ExternalInput").ap()
    maskB_d = nc.dram_tensor("maskB", [128, 512], AD, kind="ExternalInput").ap()
    out_d = nc.dram_tensor("out", [tok, C], F32, kind="ExternalOutput").ap()

    with tile.TileContext(nc) as tc, ExitStack() as ctx:
        consts = ctx.enter_context(tc.tile_pool(name="consts", bufs=1))
        xin = ctx.enter_context(tc.tile_pool(name="xin", bufs=3))
        stag = ctx.enter_context(tc.tile_pool(name="stag", bufs=1))
        smx = ctx.enter_context(tc.tile_pool(name="smx", bufs=4))
        s2p = ctx.enter_context(tc.tile_pool(name="s2p", bufs=3))
        outp = ctx.enter_context(tc.tile_pool(name="outp", bufs=3))
        psA = ctx.enter_context(tc.tile_pool(name="psA", bufs=2, space="PSUM"))
        psSp = ctx.enter_context(tc.tile_pool(name="psSp", bufs=2, space="PSUM"))
        psVp = ctx.enter_context(tc.tile_pool(name="psVp", bufs=1, space="PSUM"))
        psC2p = ctx.enter_context(tc.tile_pool(name="psC2p", bufs=1, space="PSUM"))
        psTp = ctx.enter_context(tc.tile_pool(name="psTp", bufs=1, space="PSUM"))
        psOp = ctx.enter_context(tc.tile_pool(name="psOp", bufs=1, space="PSUM"))

        # first superblock's x load goes out BEFORE the (much larger) weight
        # DMAs so the first projection matmuls aren't queued behind them
        xT_r0 = xT_d.rearrange("(ci p) t -> p ci t", p=128)
        x0_sb = xin.tile([128, 8, SB], WD, name="x0")
        nc.sync.dma_start(out=x0_sb, in_=xT_r0[:, :, bass.ds(0, SB)])

        # weights: one tile per contraction chunk so the first projection
        # matmuls only wait on their own chunk's DMA
        wq_sb = consts.tile([128, 8, 3 * C], WD)
        wq_r = wqkvT_d.rearrange("(ci p) f -> p ci f", p=128)
        for ci in range(8):
            nc.sync.dma_start(out=wq_sb[:, ci, :], in_=wq_r[:, ci, :])
        wo_sb = consts.tile([128, 8, C], WD)
        nc.sync.dma_start(out=wo_sb, in_=woutT_d.rearrange("(ci p) f -> p ci f", p=128))
        bcols_sb = consts.tile([128, 24], F32)
        nc.sync.dma_start(out=bcols_sb, in_=bcols_d)
        borep_sb = consts.tile([128, C], F32)
        nc.sync.dma_start(out=borep_sb, in_=borep_d)
        maskB_sb = consts.tile([128, 512], AD)
        nc.sync.dma_start(out=maskB_sb, in_=maskB_d)
        idf = consts.tile([128, 128], F32)
        make_identity(nc, idf)
        idr = consts.tile([128, 128], AD)
        nc.vector.tensor_copy(idr, idf)

        # persistent ping-pong staging (half-set hb=0: Q/K upper, V lower)
        Q_AB = stag.tile([128, NG, 16, 8], AD, name="Q_AB")
        K_AB = stag.tile([128, NG, 16, 8], AD, name="K_AB")
        V_AB = stag.tile([128, NG, 16, 8], AD, name="V_AB")

        xT_r = xT_d.rearrange("(ci p) t -> p ci t", p=128)

        def emit_xload(piv):
            x_sb = xin.tile([128, 8, SB], WD)
            nc.sync.dma_start(out=x_sb, in_=xT_r[:, :, bass.ds(piv, SB)])
            return x_sb

        def qk_half(T, hb):
            return T[64 * (1 - hb):64 * (1 - hb) + 64]

        def v_half(hb):
            return V_AB[64 * hb:64 * hb + 64]

        def emit_proj_chunk(x_sb, co, hb):
            psC1 = psA.tile([128, SB], F32)
            for ci in range(8):
                nc.tensor.matmul(psC1, wq_sb[:, ci, co * 128:(co + 1) * 128],
                                 x_sb[:, ci, :], start=(ci == 0), stop=(ci == 7))
            kind, c = co // 8, co % 8
            for dlt in range(2):
                src = psC1[64 * dlt:64 * dlt + 64, :].rearrange(
                    "p (g t) -> p g t", g=NG)
                hslot = 2 * c + dlt
                if kind == 0:
                    dst = qk_half(Q_AB, hb)[:, :, hslot, :]
                elif kind == 1:
                    dst = qk_half(K_AB, hb)[:, :, hslot, :]
                else:
                    dst = v_half(hb)[:, :, hslot, :]
                bias = bcols_sb[64 * dlt:64 * dlt + 64, co:co + 1]
                if dlt == 0:
                    nc.vector.tensor_scalar_add(dst, src, bias)
                else:
                    nc.scalar.activation(dst, src, Identity, bias=bias)

        def emit_attn_batch1(b4, hb):
            """scores + V transposes + exp + mask for groups 4*b4..4*b4+3."""
            psS = psSp.tile([128, 512], F32)
            psV = psVp.tile([128, 4, 64], AD)
            vb = 64 * hb
            for j in range(4):
                g = 4 * b4 + j
                nc.tensor.matmul(psS[:, 128 * j:128 * j + 128],
                                 qk_half(K_AB, hb)[:, g, :, :],
                                 qk_half(Q_AB, hb)[:, g, :, :],
                                 start=True, stop=True)
                nc.tensor.transpose(psV[:, j, :], v_half(hb)[:, g, :, :],
                                    idr[vb:vb + 64, vb:vb + 64])
            V8sb = smx.tile([128, 4, 66], AD, name="V8sb")
            nc.scalar.copy(V8sb[:, :, 0:64], psV)
            nc.gpsimd.memset(V8sb[:, :, 64:65], 1.0)
            es4 = smx.tile([128, 512], AD)
            nc.scalar.activation(es4, psS, Exp, scale=0.125)
            if MASK_ENG == "pool":
                nc.gpsimd.tensor_mul(es4, es4, maskB_sb)
            else:
                nc.vector.tensor_mul(es4, es4, maskB_sb)
            return es4, V8sb

        def emit_attn_batch2(b4, es4, V8sb, S2):
            psC2 = psC2p.tile([128, 4, 66], F32)
            for j in range(4):
                nc.tensor.matmul(psC2[:, j, 0:65], es4[:, 128 * j:128 * j + 128],
                                 V8sb[:, j, 0:65], start=True, stop=True)
            rec4 = smx.tile([128, 4], F32)
            nc.vector.reciprocal(rec4, psC2[:, :, 64:65])
            attno = smx.tile([128, 4, 64], AD)
            # normalize all 4 groups in one op: rec4 broadcast along d
            rec4b = rec4[:, :].unsqueeze(-1).broadcast_to([128, 4, 64])
            nc.vector.tensor_mul(attno, psC2[:, :, 0:64], rec4b)
            # transpose pairs of groups on PE: [128,(2,64)] -> [(2,64),128];
            # a DMA xbar transpose can't land here because the out-proj lhsT
            # needs a single-free-dim AP (BIR verifier rule)
            psT = psTp.tile([128, 2, 128], AD)
            for p2 in range(2):
                nc.tensor.transpose(
                    psT[:, p2, :],
                    attno[:, 2 * p2:2 * p2 + 2, :].rearrange("p a b -> p (a b)"),
                    idr)
            for dlt in range(2):
                for j2 in range(2):
                    src = psT[64 * j2:64 * j2 + 64, :,
                              64 * dlt:64 * dlt + 64].rearrange(
                        "p a (h t) -> p a h t", h=8)
                    dst = S2[64 * dlt:64 * dlt + 64].rearrange(
                        "p c (gb gj t) -> p gj c gb t", gb=4, gj=4)[
                            :, j2::2, :, b4 % 4, :]
                    nc.vector.tensor_copy(dst, src)

        def emit_outproj(S2, oiv, iss):
            outsb = outp.tile([128, C], F32)
            for nh in range(2):
                psO = psOp.tile([128, 512], F32)
                for c in range(8):
                    nc.tensor.matmul(psO, S2[:, c, :],
                                     wo_sb[:, c, 512 * nh:512 * nh + 512],
                                     start=(c == 0), stop=(c == 7))
                nc.vector.tensor_add(outsb[:, 512 * nh:512 * nh + 512], psO,
                                     borep_sb[:, 512 * nh:512 * nh + 512])
            nc.sync.dma_start(out=out_d[bass.ds(oiv + SS * iss, SS), :], in_=outsb)

        def emit_part(attn_oiv, attn_hb, proj_piv, proj_hb, x_pre=None,
                      pend_out=None, defer_out=False):
            """Weave attention of one superblock with projection of another.
            Either may be None (prologue/epilogue). The iss=1 out-projection
            is deferred into the next part (returned as (S2, oiv, 1)) when
            defer_out is set, so the in-order PE queue isn't blocked on the
            final S2 xbar-transpose at the part boundary."""
            if x_pre is not None:
                x_sb = x_pre
            else:
                x_sb = emit_xload(proj_piv) if proj_piv is not None else None
            S2 = V8sb = None
            for b4 in range(8):
                if b4 == 0 and x_sb is not None:
                    # boundary phase: lead with proj matmuls so the in-order
                    # PE stream has ready work while the attention chain of
                    # this part spins up.
                    for co in range(0, 3):
                        emit_proj_chunk(x_sb, co, proj_hb)
                if attn_oiv is not None:
                    if b4 % 4 == 0:
                        # S2 free layout: (c, (gb, gj, t)) — the out-proj
                        # lhsT slice S2[:, c, :] must be a single-free-dim AP
                        S2 = s2p.tile([128, 8, SS], AD, name="S2")
                    es4, V8sb = emit_attn_batch1(b4, attn_hb)
                if b4 == 0 and pend_out is not None:
                    # after scores(0): its stall window absorbs both the S2
                    # and V8 xbar-transpose latencies at the part boundary
                    emit_outproj(*pend_out)
                if x_sb is not None and b4 > 0:
                    for co in range(3 * b4, 3 * b4 + 3):
                        emit_proj_chunk(x_sb, co, proj_hb)
                if b4 == 4 and attn_oiv is not None:
                    emit_outproj(S2_0, attn_oiv, 0)
                if attn_oiv is not None:
                    emit_attn_batch2(b4, es4, V8sb, S2)
                    if b4 == 3:
                        S2_0 = S2
            if attn_oiv is None:
                return None
            if defer_out:
                return (S2, attn_oiv, 1)
            emit_outproj(S2, attn_oiv, 1)
            return None

        U = UNROLL
        nsb = tok // SB
        assert tok % SB == 0 and U % 2 == 0
        emit_part(None, None, 0, 0, x_pre=x0_sb)         # prologue: proj sb0 -> A
        nloop = max(0, nsb // U - 1)
        if nloop > 0:
            with tc.For_i(0, nloop * U * SB, U * SB,
                          hint_engines=(mybir.EngineType.PE, mybir.EngineType.DVE,
                                        mybir.EngineType.Activation)) as iv:
                pend = None
                for u in range(U):
                    pend = emit_part(iv + u * SB, u % 2, iv + (u + 1) * SB,
                                     (u + 1) % 2, pend_out=pend,
                                     defer_out=(u < U - 1))
        start = nloop * U * SB
        ntail = nsb - nloop * U
        pend = None
        for u in range(ntail):
            a = start + u * SB
            if u < ntail - 1:
                pend = emit_part(a, u % 2, a + SB, (u + 1) % 2,
                                 pend_out=pend, defer_out=True)
            else:
                emit_part(a, u % 2, None, None, pend_out=pend)  # last attn
    nc.compile()
    return nc


def _host_prep(x, w_qkv, b_qkv, w_out, b_out):
    bf = ml_dtypes.bfloat16
    d = np.arange(D)
    perm_q = (192 * np.arange(H)[:, None] + d[None, :]).reshape(-1)
    perm = np.concatenate([perm_q, perm_q + 64, perm_q + 128])
    wqkvT = np.ascontiguousarray(w_qkv[perm, :].T.astype(bf))
    bcols = np.ascontiguousarray(
        b_qkv[perm].reshape(24, 128).T, dtype=np.float32)
    # out-proj row perm: S2 row 128c+64dlt+d holds feature 64*(8dlt+c)+d
    co, dl = np.arange(8), np.arange(2)
    perm_o = (64 * (8 * dl[None, :, None] + co[:, None, None])
              + d[None, None, :]).reshape(-1)
    woutT = np.ascontiguousarray(w_out.T[perm_o, :].astype(bf))
    borep = np.ascontiguousarray(
        np.broadcast_to(b_out[None, :], (128, C)), dtype=np.float32)
    maskB = np.ascontiguousarray(np.tile(
        (np.arange(128)[:, None] % 8 == np.arange(128)[None, :] % 8)
        .astype(np.float32), (1, 4)).astype(bf))
    xT = np.ascontiguousarray(x.T.astype(bf))
    return xT, wqkvT, bcols, woutT, borep, maskB


_cache = {}


def kernel(x, w_qkv, b_qkv, w_out, b_out, _trace=False, _tmpdir=None):
    x = np.asarray(x)
    n = x.shape[0]
    tok = n // N_CORES
    xT, wqkvT, bcols, woutT, borep, maskB = _host_prep(
        np.asarray(x), np.asarray(w_qkv), np.asarray(b_qkv),
        np.asarray(w_out), np.asarray(b_out))
    key = (tok,)
    if key not in _cache:
        _cache[key] = build_pipe(tok)
    nc = _cache[key]
    shared = dict(wqkvT=wqkvT, woutT=woutT, bcols=bcols, borep=borep, maskB=maskB)
    in_maps = [dict(xT=np.ascontiguousarray(xT[:, i * tok:(i + 1) * tok]), **shared)
               for i in range(N_CORES)]
    res = run_bass_kernel_spmd(nc, in_maps, core_ids=list(range(N_CORES)),
                               trace=_trace, tmpdir=_tmpdir)
    out = np.concatenate([res.results[i]["out"] for i in range(N_CORES)], axis=0)
    kernel.last_results = res
    return out
